# revision 1
# baseline (speedup 1.0000x reference)
"""Kendall's Tau loss on 8 Trainium2 cores.

numerator = sum_{i,j} sign(p_i-p_j)*sign(t_i-t_j) / 2.  We compute
prod[i,j] = (p_i-p_j)*(t_i-t_j) = a_i + a_j - p_i*t_j - t_i*p_j  (a = p*t)
as a K=18 bf16 matmul on the TensorEngine (each fp32 operand split into
3 bf16 terms, small cross terms dropped -> ~fp32 accuracy), then
sign+reduce on ScalarE (Sign activation with accum_out) and VectorE
(is_gt/is_lt counting), using the symmetry S[i,j]=S[j,i] to do only the
upper block-triangle.  Diagonal 128x128 blocks are handled separately
with a mask that zeroes i==j (where the expansion leaves fp noise
instead of an exact 0).  Host sums the per-core accumulator cells.
"""
import sys

sys.path.insert(0, "/opt/trn_rl_repo")

import numpy as np
import ml_dtypes

import concourse.bass as bass
from concourse import mybir
from concourse.bass_utils import run_bass_kernel_spmd

BF16 = ml_dtypes.bfloat16
N = 8192
NB = 64          # 128-row blocks
NCORES = 8
K = 18           # rank of the product expansion
NRUNS = 66       # strip runs of 512 cols (4 x 128-col blocks) per core
DVE_SET = (2, 5, 8, 11, 14, 16)   # psum tensors reduced on VectorE
ACT_LIST = tuple(ti for ti in range(17) if ti not in DVE_SET)  # 11 on ScalarE
NCELL = 24       # 11 ACT + 6 gt + 6 lt + 1 diag


def _split3(x64):
    h = x64.astype(BF16)
    r = x64 - h.astype(np.float64)
    m = r.astype(BF16)
    l = (r - m.astype(np.float64)).astype(BF16)
    return h, m, l


def _core_rows(k):
    return [4 * k, 4 * k + 1, 4 * k + 2, 4 * k + 3,
            60 - 4 * k, 61 - 4 * k, 62 - 4 * k, 63 - 4 * k]


def _build_inputs(p, t):
    p64 = p.astype(np.float64)
    t64 = t.astype(np.float64)
    ph, pm, pl = _split3(p64)
    th, tm, tl = _split3(t64)
    ah, am, al = _split3(p64 * t64)
    one = np.ones(N, dtype=BF16)
    L = np.stack([ah, am, al, one, one, one,
                  -ph, -ph, -ph, -pm, -pm, -pl,
                  -th, -th, -th, -tm, -tm, -tl])
    R = np.stack([one, one, one, ah, am, al,
                  th, tm, tl, th, tm, th,
                  ph, pm, pl, ph, pm, ph])
    mask = np.ones((128, 8 * 128), dtype=BF16)
    for d in range(8):
        mask[np.arange(128), d * 128 + np.arange(128)] = 0

    in_maps = []
    for k in range(NCORES):
        rows = _core_rows(k)
        runs = []
        for r in rows:
            qs = list(range(r + 1, NB))
            for i in range(0, len(qs), 4):
                grp = qs[i:i + 4]
                grp += [None] * (4 - len(grp))
                runs.append((r, grp))
        assert len(runs) == NRUNS, (k, len(runs))
        lhsw = np.zeros((K, NRUNS * 128), dtype=BF16)
        rhsseq = np.zeros((K, NRUNS * 512), dtype=BF16)
        for m, (r, grp) in enumerate(runs):
            lhsw[:, m * 128:(m + 1) * 128] = L[:, r * 128:(r + 1) * 128]
            for s, q in enumerate(grp):
                if q is not None:
                    rhsseq[:, m * 512 + s * 128: m * 512 + (s + 1) * 128] = \
                        R[:, q * 128:(q + 1) * 128]
        ldiag = np.concatenate(
            [L[:, r * 128:(r + 1) * 128] for r in rows], axis=1)
        rdiag = np.concatenate(
            [R[:, r * 128:(r + 1) * 128] for r in rows], axis=1)
        in_maps.append({"lhsw": lhsw, "rhsseq": rhsseq,
                        "ldiag": ldiag, "rdiag": rdiag, "mask": mask})
    return in_maps


_NC_CACHE = []


def _build_nc():
    # Cross-engine deps are fully semaphore-ordered by construction; the
    # remaining WAW on scratch ("trash") buffers is same-engine in-order
    # and safe on HW, but trips the sim's conservative race detector.
    nc = bass.Bass(detect_race_conditions=False)
    dt = mybir.dt
    lhsw_d = nc.dram_tensor("lhsw", [K, NRUNS * 128], dt.bfloat16,
                            kind="ExternalInput")
    rhs_d = nc.dram_tensor("rhsseq", [K, NRUNS * 512], dt.bfloat16,
                           kind="ExternalInput")
    ldiag_d = nc.dram_tensor("ldiag", [K, 1024], dt.bfloat16,
                             kind="ExternalInput")
    rdiag_d = nc.dram_tensor("rdiag", [K, 1024], dt.bfloat16,
                             kind="ExternalInput")
    mask_d = nc.dram_tensor("mask", [128, 1024], dt.bfloat16,
                            kind="ExternalInput")
    acc_d = nc.dram_tensor("acc_out", [128, NCELL], dt.float32,
                           kind="ExternalOutput")

    # signred engine + completion ordinal for each strip tensor
    sr_of = {}
    for i, ti in enumerate(ACT_LIST):
        sr_of[ti] = ("act", i + 1)
    for i, ti in enumerate(sorted(DVE_SET)):
        sr_of[ti] = ("dve", i + 1)

    with (
        nc.sbuf_tensor([K, NRUNS * 128], dt.bfloat16) as lhsw_s,
        nc.sbuf_tensor([K, NRUNS * 512], dt.bfloat16) as rhs_s,
        nc.sbuf_tensor([K, 1024], dt.bfloat16) as ldiag_s,
        nc.sbuf_tensor([K, 1024], dt.bfloat16) as rdiag_s,
        nc.sbuf_tensor([128, 1024], dt.bfloat16) as mask_s,
        nc.sbuf_tensor([128, NCELL], dt.float32) as acc_s,
        nc.sbuf_tensor([128, 2048], dt.bfloat16) as trash_a,
        nc.sbuf_tensor([128, 2048], dt.bfloat16) as trash_v,
        nc.sbuf_tensor([128, 1024], dt.bfloat16) as sgn_s,
        nc.sbuf_tensor([128, 1024], dt.bfloat16) as mprod_s,
        nc.sbuf_tensor([128, 1], dt.float32) as dummy,
        nc.sbuf_tensor([128, 1], dt.bfloat16) as dummy_o,
        nc.psum_tensor([128, 2048], dt.float32) as ps0,
        nc.psum_tensor([128, 2048], dt.float32) as ps1,
        nc.semaphore("dma_sem") as dma_sem,
        nc.semaphore("sem_early") as sem_early,
        nc.semaphore("sem_mm") as sem_mm,
        nc.semaphore("sem_act") as sem_act,
        nc.semaphore("sem_dve") as sem_dve,
        nc.semaphore("sem_misc") as sem_misc,
        nc.Block() as block,
    ):
        ps = [ps0, ps1]

        @block.gpsimd
        def _(g):
            g.memset(dummy[:], 0.0).then_inc(sem_misc, 1)

        @block.sync
        def _(sync):
            half = 16 * 512
            sync.dma_start(lhsw_s[:], lhsw_d[:]).then_inc(sem_early, 16)
            sync.dma_start(rhs_s[:, :half], rhs_d[:, :half]).then_inc(sem_early, 16)
            sync.dma_start(rhs_s[:, half:], rhs_d[:, half:]).then_inc(dma_sem, 16)
            sync.dma_start(ldiag_s[:], ldiag_d[:]).then_inc(dma_sem, 16)
            sync.dma_start(rdiag_s[:], rdiag_d[:]).then_inc(dma_sem, 16)
            sync.dma_start(mask_s[:], mask_d[:]).then_inc(dma_sem, 16)
            sync.wait_ge(sem_act, len(ACT_LIST) + 1)
            sync.wait_ge(sem_dve, len(DVE_SET) + 1)
            sync.dma_start(acc_d[:], acc_s[:]).then_inc(dma_sem, 16)

        @block.tensor
        def _(te):
            te.wait_ge(sem_early, 32)
            for ti in range(17):
                if ti == 4:
                    te.wait_ge(dma_sem, 64)
                if ti >= 2:
                    eng, cnt = sr_of[ti - 2]
                    te.wait_ge(sem_act if eng == "act" else sem_dve, cnt)
                fd = 2048 if ti < 16 else 1024
                for j in range(fd // 512):
                    run = ti * 4 + j
                    mm = nc.tensor.matmul(
                        ps[ti % 2][:, j * 512:(j + 1) * 512],
                        lhsw_s[:, run * 128:(run + 1) * 128],
                        rhs_s[:, run * 512:(run + 1) * 512],
                        start=True, stop=True)
                    if j == fd // 512 - 1:
                        mm.then_inc(sem_mm, 1)
            # diag blocks = "tensor 17", into ps1 (2 per bank, stride 256)
            eng, cnt = sr_of[15]
            te.wait_ge(sem_act if eng == "act" else sem_dve, cnt)
            for d in range(8):
                mm = nc.tensor.matmul(
                    ps[1][:, d * 256:d * 256 + 128],
                    ldiag_s[:, d * 128:(d + 1) * 128],
                    rdiag_s[:, d * 128:(d + 1) * 128],
                    start=True, stop=True)
                if d == 7:
                    mm.then_inc(sem_mm, 1)

        @block.scalar
        def _(sc):
            sc.wait_ge(sem_misc, 1)
            nc.scalar.activation(dummy_o[:], dummy[:],
                                 mybir.ActivationFunctionType.Sign)
            ai = 0
            for ti in ACT_LIST:
                sc.wait_ge(sem_mm, ti + 1)
                fd = 2048 if ti < 16 else 1024
                nc.scalar.activation(
                    trash_a[:, :fd], ps[ti % 2][:, :fd],
                    mybir.ActivationFunctionType.Sign,
                    accum_out=acc_s[:, ai:ai + 1]).then_inc(sem_act, 1)
                ai += 1
            sc.wait_ge(sem_mm, 18)
            psd = ps[1][:, 0:2048].rearrange("p (a b) -> p a b", b=256)[:, :, 0:128]
            sgv = sgn_s[:, 0:1024].rearrange("p (a b) -> p a b", b=128)
            nc.scalar.activation(sgv, psd,
                                 mybir.ActivationFunctionType.Sign
                                 ).then_inc(sem_act, 1)

        @block.vector
        def _(ve):
            di = 0
            for ti in sorted(DVE_SET):
                ve.wait_ge(sem_mm, ti + 1)
                fd = 2048 if ti < 16 else 1024
                nc.vector.tensor_scalar(
                    trash_v[:, :fd], ps[ti % 2][:, :fd], 0.0, None,
                    mybir.AluOpType.is_gt, op1=mybir.AluOpType.add,
                    accum_out=acc_s[:, 11 + di:12 + di])
                nc.vector.tensor_scalar(
                    trash_v[:, :fd], ps[ti % 2][:, :fd], 0.0, None,
                    mybir.AluOpType.is_lt, op1=mybir.AluOpType.add,
                    accum_out=acc_s[:, 17 + di:18 + di]).then_inc(sem_dve, 1)
                di += 1
            ve.wait_ge(sem_act, len(ACT_LIST) + 1)
            nc.vector.tensor_mul(mprod_s[:, :1024], sgn_s[:, :1024],
                                 mask_s[:, :1024])
            nc.vector.tensor_scalar(
                trash_v[:, :1024], mprod_s[:, :1024], 0.0, None,
                mybir.AluOpType.add, op1=mybir.AluOpType.add,
                accum_out=acc_s[:, 23:24]).then_inc(sem_dve, 1)

    return nc


def _get_nc():
    if not _NC_CACHE:
        _NC_CACHE.append(_build_nc())
    return _NC_CACHE[0]


def kernel(predictions, true_labels, _trace=False):
    p = np.asarray(predictions, dtype=np.float32)
    t = np.asarray(true_labels, dtype=np.float32)
    in_maps = _build_inputs(p, t)
    nc = _get_nc()
    res = run_bass_kernel_spmd(nc, in_maps, list(range(NCORES)), trace=_trace)
    total = 0.0
    for k in range(NCORES):
        acc = res.results[k]["acc_out"].astype(np.float64)
        strip = acc[:, 0:11].sum() + acc[:, 11:17].sum() - acc[:, 17:23].sum()
        total += 2.0 * strip + acc[:, 23].sum()
    loss = 1.0 - total / (N * (N - 1))
    out = np.array(loss, dtype=np.float32)
    if _trace:
        return out, res
    return out



# revision 2
# speedup vs baseline: 1.0699x; 1.0699x over previous
"""Kendall's Tau loss on 8 Trainium2 cores.

numerator = sum_{i,j} sign(p_i-p_j)*sign(t_i-t_j) / 2.  We compute
prod[i,j] = (p_i-p_j)*(t_i-t_j) = a_i + a_j - p_i*t_j - t_i*p_j  (a = p*t)
as a K=10 bf16 matmul on the TensorEngine (fp32 operands 2-split into
bf16 high/low terms, the low*low cross terms dropped -> ~1e-5 abs error),
then reduce the sign of each pairwise product in a single pass per
element, spread over three engines:

  - ScalarE:  Sign activation with accum_out (direct sign-sum)
  - VectorE:  tensor_scalar is_lt 0 with accum_out (negative count)
  - Pool:     tensor_tensor is_lt -> bf16 0/1 flags in SBUF, then VectorE
              sums the flags in 4x DVE mode (negative count)

For count-based cells the host reconstructs sum(sign) = total - 2*negs
(exact zeros are rare and well inside the accuracy budget; the diagonal
i==j leaves only fp noise whose sign contributes < 8k of a ~671k error
budget, so no masking).

Each core owns 8 of the 64 row-blocks and processes its upper-triangle
strip (plus diagonal blocks, which the host counts once instead of
twice).  Work is a flat stream of 260 128-col blocks; the host packs
per-core lhs/rhs streams so the bass program is identical across cores.
Dummy matmuls on scratch SBUF warm the PE p-state ramp while the input
DMA lands, so the real stream runs at full clock with no PE idles.
"""
import sys

sys.path.insert(0, "/opt/trn_rl_repo")

import numpy as np
import ml_dtypes

import concourse.bass as bass
from concourse import mybir
from concourse.bass_utils import run_bass_kernel_spmd

BF16 = ml_dtypes.bfloat16
N = 8192
NB = 64            # 128-row blocks
NCORES = 8
K = 10             # rank of the product expansion
NBLK = 260         # 8 diag + 252 strip blocks per core
SCOLS = NBLK * 128 # stream columns per core
RING = 32          # PSUM ring: 32 blocks of 128 = 4096 fp32 cols
WARM_MM = 34       # dummy matmuls to ramp the PE before the stream
WARM_GATE = 31     # dummy index at which PE waits for the first DMAs
# input DMA pieces (stream column ranges); piece 1 gates the PE start
PIECES = ((0, 8192), (8192, 20736), (20736, SCOLS))

# consumer chunks: (n_blocks, engine); chunk 0 is the 8 diag blocks.
# A=ScalarE sign-accum, D=VectorE is_lt-accum, P=Pool flags + DVE reduce.
# Greedy load-balance with modeled per-chunk costs (ns).
_COST = {"A": 1215.0, "D": 1199.0, "P": 900.0}
_COST_PRED = 350.0   # DVE reduce of a Pool flag chunk


def _schedule():
    sizes = [8] * 32 + [4]
    loads = {"A": 0.0, "D": 0.0, "P": 0.0}
    sched = []
    for c, nb in enumerate(sizes):
        scale = nb / 8.0
        if c == 0:
            e = "A"
        else:
            best = None
            for cand in ("A", "D", "P"):
                trial = dict(loads)
                trial[cand] += _COST[cand] * scale
                if cand == "P":
                    trial["D"] += _COST_PRED * scale
                m = max(trial.values())
                if best is None or m < best[0]:
                    best = (m, cand)
            e = best[1]
        loads[e] += _COST[e] * (nb / 8.0)
        if e == "P":
            loads["D"] += _COST_PRED * (nb / 8.0)
        sched.append((nb, e))
    assert sum(nb for nb, _ in sched) == NBLK
    return sched


SCHED = _schedule()
NCHUNK = len(SCHED)


def _core_rows(k):
    return [4 * k, 4 * k + 1, 4 * k + 2, 4 * k + 3,
            60 - 4 * k, 61 - 4 * k, 62 - 4 * k, 63 - 4 * k]


def _core_blocks(k):
    rows = _core_rows(k)
    blocks = [(r, r) for r in rows]
    for r in rows:
        blocks.extend((r, q) for q in range(r + 1, NB))
    assert len(blocks) == NBLK
    return blocks


def _split2(x64):
    h = x64.astype(BF16)
    l = (x64 - h.astype(np.float64)).astype(BF16)
    return h, l


def _build_inputs(p, t):
    p64 = p.astype(np.float64)
    t64 = t.astype(np.float64)
    ph, pl = _split2(p64)
    th, tl = _split2(t64)
    ah, al = _split2(p64 * t64)
    one = np.ones(N, dtype=BF16)
    L = np.stack([ah, al, one, one, -ph, -ph, -pl, -th, -th, -tl])
    R = np.stack([one, one, ah, al, th, tl, th, ph, pl, ph])
    L3 = np.ascontiguousarray(L.reshape(K, NB, 128))
    R3 = np.ascontiguousarray(R.reshape(K, NB, 128))

    in_maps = []
    for k in range(NCORES):
        blocks = _core_blocks(k)
        ridx = np.array([r for r, _ in blocks])
        qidx = np.array([q for _, q in blocks])
        lst = L3[:, ridx, :].reshape(K, SCOLS)
        rst = R3[:, qidx, :].reshape(K, SCOLS)
        in_maps.append({"lstream": lst, "rstream": rst})
    return in_maps


_NC_CACHE = []


def _build_nc():
    # Cross-engine deps are fully semaphore-ordered by construction; the
    # remaining WAW on scratch ("trash") buffers is same-engine in-order
    # and safe on HW, but trips the sim's conservative race detector.
    nc = bass.Bass(detect_race_conditions=False)
    dt = mybir.dt
    lst_d = nc.dram_tensor("lstream", [K, SCOLS], dt.bfloat16,
                           kind="ExternalInput")
    rst_d = nc.dram_tensor("rstream", [K, SCOLS], dt.bfloat16,
                           kind="ExternalInput")
    acc_d = nc.dram_tensor("acc_out", [128, NCHUNK], dt.float32,
                           kind="ExternalOutput")

    # per-chunk bookkeeping for semaphores
    chunk_first = []   # first block index of each chunk
    chunk_last = []
    pos = 0
    for nb, _ in SCHED:
        chunk_first.append(pos)
        chunk_last.append(pos + nb - 1)
        pos += nb
    # ring sem identity: which (engine sem, count) frees each chunk's psum
    eng_count = {"A": 0, "D": 0, "P": 0}
    ring_free = []     # (engine, count_after_consume)
    pool_seq = []      # running index of pool chunks, by chunk id
    for nb, e in SCHED:
        eng_count[e] += 1
        ring_free.append((e, eng_count[e]))
        pool_seq.append(eng_count["P"] - 1 if e == "P" else None)
    n_act = eng_count["A"]
    n_dve = eng_count["D"]
    n_pool = eng_count["P"]

    with (
        nc.sbuf_tensor([K, SCOLS], dt.bfloat16) as lst_s,
        nc.sbuf_tensor([K, SCOLS], dt.bfloat16) as rst_s,
        nc.sbuf_tensor([K, 256], dt.bfloat16) as warm_s,
        nc.sbuf_tensor([128, 1024], dt.bfloat16) as zeros_s,
        nc.sbuf_tensor([128, 4 * 1024], dt.bfloat16) as flags_s,
        nc.sbuf_tensor([128, 1024], dt.bfloat16) as trash_a,
        nc.sbuf_tensor([128, 1024], dt.bfloat16) as trash_v,
        nc.sbuf_tensor([128, 1024], dt.bfloat16) as trash_f,
        nc.sbuf_tensor([128, NCHUNK], dt.float32) as acc_s,
        nc.sbuf_tensor([128, 1], dt.float32) as dummy,
        nc.sbuf_tensor([128, 1], dt.bfloat16) as dummy_o,
        nc.psum_tensor([128, 2048], dt.float32) as ps0,
        nc.psum_tensor([128, 2048], dt.float32) as ps1,
        nc.semaphore("dma_sem") as dma_sem,
        nc.semaphore("sem_misc") as sem_misc,
        nc.semaphore("sem_mm") as sem_mm,
        nc.semaphore("sem_act") as sem_act,
        nc.semaphore("sem_dve") as sem_dve,
        nc.semaphore("sem_pool") as sem_pool,
        nc.semaphore("sem_dred") as sem_dred,
        nc.Block() as block,
    ):
        def ps_ap(blk, nblk=1):
            # stream block -> psum ring slot ((blk % 32) * 128 columns)
            s = (blk % RING) * 128
            t, o = (ps0, s) if s < 2048 else (ps1, s - 2048)
            return t[:, o:o + 128 * nblk]

        @block.sync
        def _(sync):
            for lo, hi in PIECES:
                sync.dma_start(lst_s[:, lo:hi],
                               lst_d[:, lo:hi]).then_inc(dma_sem, 16)
                sync.dma_start(rst_s[:, lo:hi],
                               rst_d[:, lo:hi]).then_inc(dma_sem, 16)
            sync.wait_ge(sem_act, n_act)
            sync.wait_ge(sem_dve, n_dve)
            sync.wait_ge(sem_dred, n_pool)
            sync.dma_start(acc_d[:], acc_s[:]).then_inc(dma_sem, 16)

        @block.gpsimd
        def _(g):
            g.memset(dummy[:], 0.0)
            g.memset(warm_s[:], 0.0)
            g.memset(zeros_s[:], 0.0).then_inc(sem_misc, 1)
            pi = 0
            for c, (nb, e) in enumerate(SCHED):
                if e != "P":
                    continue
                g.wait_ge(sem_mm, c + 1)
                if pi >= 4:
                    g.wait_ge(sem_dred, pi - 3)
                fo = (pi % 4) * 1024
                nc.gpsimd.tensor_tensor(
                    flags_s[:, fo:fo + nb * 128],
                    ps_ap(chunk_first[c], nb),
                    zeros_s[:, :nb * 128],
                    mybir.AluOpType.is_lt).then_inc(sem_pool, 1)
                pi += 1

        @block.tensor
        def _(te):
            te.wait_ge(sem_misc, 1)
            for w in range(WARM_MM):
                if w == WARM_GATE:
                    te.wait_ge(dma_sem, 32)
                nc.tensor.matmul(ps0[:, 0:128], warm_s[:, 0:128],
                                 warm_s[:, 128:256], start=True, stop=True)
            c = 0
            for m in range(NBLK):
                if m == chunk_first[c]:
                    if c >= 4:
                        e, cnt = ring_free[c - 4]
                        sem = {"A": sem_act, "D": sem_dve,
                               "P": sem_pool}[e]
                        te.wait_ge(sem, cnt)
                if m == 64:
                    te.wait_ge(dma_sem, 64)
                if m == 162:
                    te.wait_ge(dma_sem, 96)
                mm = nc.tensor.matmul(
                    ps_ap(m),
                    lst_s[:, m * 128:(m + 1) * 128],
                    rst_s[:, m * 128:(m + 1) * 128],
                    start=True, stop=True)
                if m == chunk_last[c]:
                    mm.then_inc(sem_mm, 1)
                    c += 1

        @block.scalar
        def _(sc):
            sc.wait_ge(sem_misc, 1)
            nc.scalar.activation(dummy_o[:], dummy[:],
                                 mybir.ActivationFunctionType.Sign)
            ai = 0
            for c, (nb, e) in enumerate(SCHED):
                if e != "A":
                    continue
                sc.wait_ge(sem_mm, c + 1)
                nc.scalar.activation(
                    trash_a[:, :nb * 128], ps_ap(chunk_first[c], nb),
                    mybir.ActivationFunctionType.Sign,
                    accum_out=acc_s[:, c:c + 1]).then_inc(sem_act, 1)
                ai += 1

        @block.vector
        def _(ve):
            pi = 0
            for c, (nb, e) in enumerate(SCHED):
                if e == "D":
                    ve.wait_ge(sem_mm, c + 1)
                    nc.vector.tensor_scalar(
                        trash_v[:, :nb * 128], ps_ap(chunk_first[c], nb),
                        0.0, None,
                        mybir.AluOpType.is_lt, op1=mybir.AluOpType.add,
                        accum_out=acc_s[:, c:c + 1]).then_inc(sem_dve, 1)
                elif e == "P":
                    ve.wait_ge(sem_pool, pi + 1)
                    fo = (pi % 4) * 1024
                    nc.vector.tensor_scalar(
                        trash_f[:, :nb * 128],
                        flags_s[:, fo:fo + nb * 128], 0.0, None,
                        mybir.AluOpType.add, op1=mybir.AluOpType.add,
                        accum_out=acc_s[:, c:c + 1]).then_inc(sem_dred, 1)
                    pi += 1

    return nc


def _get_nc():
    if not _NC_CACHE:
        _NC_CACHE.append(_build_nc())
    return _NC_CACHE[0]


def kernel(predictions, true_labels, _trace=False):
    p = np.asarray(predictions, dtype=np.float32)
    t = np.asarray(true_labels, dtype=np.float32)
    in_maps = _build_inputs(p, t)
    nc = _get_nc()
    res = run_bass_kernel_spmd(nc, in_maps, list(range(NCORES)), trace=_trace)
    total = 0.0
    for k in range(NCORES):
        acc = res.results[k]["acc_out"].astype(np.float64)
        cell = acc.sum(axis=0)
        for c, (nb, e) in enumerate(SCHED):
            s = cell[c] if e == "A" else nb * 128 * 128 - 2.0 * cell[c]
            total += s if c == 0 else 2.0 * s
    loss = 1.0 - total / (N * (N - 1))
    out = np.array(loss, dtype=np.float32)
    if _trace:
        return out, res
    return out
